# revision 16
# baseline (speedup 1.0000x reference)
"""Trainium2 Bass kernel for DenseCapsule dynamic routing (3 iterations).

Problem: x[128,2048,8] f32, weight[16,2048,16,8] f32 -> out[128,16,16] f32.
  x_hat = einsum('oide,bie->boid', W, x); 3 routing iterations
  (softmax over o, c-weighted i-sum, squash, agreement update).

Strategy (8 NeuronCores, shard in_num_caps I=2048 -> 256 per core):
  x_hat is never materialized. Per iteration, everything factors through W:
    iter1: c uniform -> s1 = (1/16) * [x @ W]   (one matmul chain)
    u = v . W (PE), l = sum_e x*u (DVE), softmax (ACT/DVE),
    xc = c*x (DVE), s = xc @ W (PE).
  Cross-core: AllReduce of partial s ([128,16,16] f32) after iters 1 and 2;
  the final iteration's partial s is returned per-core and the host does the
  gather-sum + final squash (that is the "unshard" step).

Performance changes vs the first working version (in-kernel 196 -> 176.9us,
per-core pre-AllReduce path 27.6 -> 21.5us; measured total also includes a
30-63us run-variable launch-skew gate at the first AllReduce -- the profiled
core waits for the last-dispatched core):
  - single ACT table set: sqrt(n2) = exp(0.5*ln(n2)) and a pre-placed
    InstLoadActFuncSet(natural_log_exp_and_others) -> 1 table load
    instead of 5 (1.28us each, 3 on the critical path).
  - input loads split across DMA queues (one dma_start rides one queue;
    the monolithic 1MB wbf load was a 7.3us single-queue serial tail).
  - HAM warm-up matmuls gated on each AllReduce result fill the squash
    windows so the ul chains run at 2.4GHz instead of the cold 1.2GHz.
  - softmax exp quarters emitted inside the ul loop (ACT queue is FIFO:
    emitted after the loop they queued behind all 16 u-evacuations).
  - bf16 AllReduce payloads (comms 14.8->12.4us, 13.5->10.6us).
  - balanced bf16 Z-tree (5 ops) instead of split f32 trees (~9 ops);
    reciprocal_approx_fast for 1/Z.
  - xu/tree/xc DVE ops batched over o-pairs; pair 0 split per-o so the
    first xu starts after the first PSUM evacuation.
  - iter-3 logit accumulation (b += l) folded per-pair into the tree flow.
  - s-halves evacuated + DMA'd as soon as o0-7 complete (earlier
    AllReduce trigger / output streaming).

Layout conventions per core (SBUF partition dim first):
  i_local = ih*128 + il  (ih in {0,1}, il = partition 0..127)
  o = 4*h + g            (g in 0..3 selects a 32-partition group, h in 0..3)
  d padded to 32 rows (dd) for the u-matmul stationary operand.
"""

import sys

for _p in ("/opt/trn_rl_repo", "/root/.axon_site/_ro/trn_rl_repo"):
    if _p not in sys.path:
        sys.path.insert(0, _p)

import numpy as np
import ml_dtypes

import concourse.bass as bass
import concourse.bacc as bacc
import concourse.mybir as mybir
import concourse.tile as tile
from concourse.bass_utils import run_bass_kernel_spmd

F32 = mybir.dt.float32
BF16 = mybir.dt.bfloat16
NPBF16 = ml_dtypes.bfloat16
EXP = mybir.ActivationFunctionType.Exp
LN = mybir.ActivationFunctionType.Ln
SQUARE = mybir.ActivationFunctionType.Square

N_CORES = 8
B = 128          # batch
I_FULL = 2048    # in caps
IC = 256         # in caps per core
IL = 128         # partition dim of i
IH = IC // IL    # 2
E = 8            # in cap dim
O = 16           # out caps
D = 16           # out cap dim
EPS = 1e-8

_CACHE = {}


def _emit_squash(nc, pool, sfull, vpad, tag):
    """squash on [(b)=128, (h,g,d)=256] f32 layout; writes v into vpad
    ([(b), (h,g,dd=32)=512] f32, pad rows stay zero).

    scale = n2/((1+n2)(n+eps)) ~= n/(1+n2) with n = sqrt(n2); sqrt is
    computed as exp(0.5*ln(n2)) to stay inside one ACT table set."""
    sq = pool.tile([B, O * D], F32, tag="sq")
    nc.scalar.square(sq[:, :], sfull[:, :])
    nrm2 = pool.tile([B, O], F32, tag="nrm2")
    # reduce innermost d (16) of (o=16, d=16)
    nc.vector.reduce_sum(
        nrm2[:, :],
        sq[:, :].rearrange("p (o d) -> p o d", d=D),
        axis=mybir.AxisListType.X,
    )
    # n = sqrt(n2) = exp(0.5 * ln(n2)); ln(0) = -inf -> exp(-inf) = 0  (ok)
    lnn = pool.tile([B, O], F32, tag="lnn")
    nc.scalar.activation(lnn[:, :], nrm2[:, :], LN)
    q = pool.tile([B, O], F32, tag="q")
    nc.scalar.activation(q[:, :], lnn[:, :], EXP, scale=0.5)
    t1 = pool.tile([B, O], F32, tag="t1")
    nc.vector.tensor_scalar_add(t1[:, :], nrm2[:, :], 1.0)
    rden = pool.tile([B, O], F32, tag="rden")
    nc.vector.reciprocal(rden[:, :], t1[:, :])
    scale = pool.tile([B, O], F32, tag="scale")
    nc.vector.tensor_mul(scale[:, :], q[:, :], rden[:, :])
    # v = s * scale (broadcast over d) into vpad[(b), (h, g, dd<16)]
    s_v = sfull[:, :].rearrange("p (h g d) -> p h g d", h=4, g=4)
    scale_v = scale[:, :].rearrange("p (h g) -> p h g", h=4).broadcast_to(
        (B, 4, 4, D)
    )
    vslice = vpad[:, :].rearrange("p (h g dd) -> p h g dd", h=4, g=4)[:, :, :, 0:D]
    nc.vector.tensor_tensor(vslice, s_v, scale_v, op=mybir.AluOpType.mult)


def _emit_transpose_v(nc, psum_pool, pool, vpad, vT, ident, tag):
    """vpad [(b), (h, g, dd)=512] f32 -> vT [(g,dd)=128, (h,b)=512] bf16
    via 4 PE transposes (one per h) + ACT evacuations."""
    for h in range(4):
        tp = psum_pool.tile([128, B], F32, tag="ps")
        in_slice = vpad[:, h * 128:(h + 1) * 128]
        nc.tensor.transpose(tp[:, :], in_slice, ident[:, :])
        nc.scalar.copy(vT[:, h * B:(h + 1) * B], tp[:, :])


def _emit_iteration_ul(nc, tc, pools, vT, l_buf, delta_buf, wdt, xbf, itr,
                       exp_buf):
    """u = v.W (PE) -> evac (ACT) -> per o-PAIR: xu = x*u (DVE) ->
    e-reduction rounds (DVE) -> l (or delta for iter 3)."""
    pool, psum_pool, seq = pools
    PAIR = 2 * IH * E * B  # 4096: free extent of one o-pair
    for op_ in range(O // 2):  # o-pair index
        u2 = pool.tile([IL, PAIR], BF16, tag="u2")
        for oo in range(2):
            o = op_ * 2 + oo
            h, g = o // 4, o % 4
            u_ps = psum_pool.tile([IL, IH * E * B], F32, tag="ps")
            for ih in range(IH):
                for e in range(E):
                    lhsT = wdt[:, :].rearrange(
                        "p (h ih e il) -> p h ih e il", h=4, ih=IH, e=E
                    )[32 * g:32 * (g + 1), h, ih, e, :]
                    rhs = vT[32 * g:32 * (g + 1), h * B:(h + 1) * B]
                    nc.tensor.matmul(
                        u_ps[:, (ih * E + e) * B:(ih * E + e + 1) * B], lhsT, rhs,
                        start=True, stop=True, tile_position=(32 * g, 0),
                    )
            if op_ == 3:
                nc.vector.tensor_copy(
                    u2[:, oo * IH * E * B:(oo + 1) * IH * E * B], u_ps[:, :])
            else:
                nc.scalar.copy(u2[:, oo * IH * E * B:(oo + 1) * IH * E * B],
                               u_ps[:, :])
        # xu = x * u; pair 0 is split per-o so the first xu starts right
        # after the first evacuation instead of waiting for both
        xu = pool.tile([IL, PAIR], BF16, tag="xu")
        HALF_P = IH * E * B
        xsub = 2 if op_ == 0 else 1
        for _s in range(xsub):
            lo = _s * (2 // xsub)
            n_oo = 2 // xsub
            nc.vector.tensor_tensor(
                xu[:, lo * HALF_P:(lo + n_oo) * HALF_P]
                .rearrange("p (oo ihe b) -> p oo ihe b", oo=n_oo, b=B),
                xbf[:, :].rearrange("p (ihe b) -> p ihe b", b=B)
                .unsqueeze(1).broadcast_to((IL, n_oo, IH * E, B)),
                u2[:, lo * HALF_P:(lo + n_oo) * HALF_P]
                .rearrange("p (oo ihe b) -> p oo ihe b", oo=n_oo, b=B),
                op=mybir.AluOpType.mult,
            )
        # e-reduction rounds within each (oo, ih) block: 8 -> 4 -> 2 -> 1
        xu4 = xu[:, :].rearrange("p (x half eb) -> p x half eb",
                                 x=2 * IH, half=2)
        r1 = pool.tile([IL, PAIR // 2], BF16, tag="r1")
        r1v = r1[:, :].rearrange("p (x eb) -> p x eb", x=2 * IH)
        for _s in range(xsub):
            sl = slice(_s * (2 * IH // xsub), (_s + 1) * (2 * IH // xsub))
            nc.vector.tensor_tensor(r1v[:, sl], xu4[:, sl, 0], xu4[:, sl, 1],
                                    op=mybir.AluOpType.add)
        r1h = r1[:, :].rearrange("p (x half eb) -> p x half eb",
                                 x=2 * IH, half=2)
        r2 = pool.tile([IL, PAIR // 4], BF16, tag="r2")
        r2v = r2[:, :].rearrange("p (x eb) -> p x eb", x=2 * IH)
        nc.vector.tensor_tensor(r2v, r1h[:, :, 0], r1h[:, :, 1],
                                op=mybir.AluOpType.add)
        r2h = r2[:, :].rearrange("p (x half b) -> p x half b",
                                 x=2 * IH, half=2)
        dst_buf = l_buf if itr == 2 else delta_buf
        dst = dst_buf[:, :].rearrange(
            "p (op x b) -> p op x b", op=O // 2, x=2 * IH
        )[:, op_]
        nc.vector.tensor_tensor(dst, r2h[:, :, 0], r2h[:, :, 1],
                                op=mybir.AluOpType.add)
        if itr == 3:
            # fold b += l per pair (pipelines; unblocks exp sooner)
            lp = l_buf[:, op_ * 2 * IH * B:(op_ + 1) * 2 * IH * B]
            dp = delta_buf[:, op_ * 2 * IH * B:(op_ + 1) * 2 * IH * B]
            nc.vector.tensor_add(lp, lp, dp)
        if op_ % 2 == 1:
            _q = op_ // 2
            QW = 4 * IH * B
            nc.scalar.activation(exp_buf[:, _q * QW:(_q + 1) * QW],
                                 l_buf[:, _q * QW:(_q + 1) * QW], EXP)


def _emit_softmax_xc_s(nc, tc, pools, exp_buf, xbf, wbf, s_ps_list, itr,
                       half_cb=None):
    """exp (ACT, halves), balanced bf16 Z-tree, 1/Z (fast recip),
    xprime = x/Z, then per o-pair xc = exp*xprime and the 16 accumulating
    s-matmuls per o into s_ps halves [(b), 8*16] each."""
    pool, psum_pool, seq = pools
    HALF = 8 * IH * B  # 2048
    Q = HALF // 2      # 1024: one exp quarter (4 o's)
    za = seq.tile([IL, HALF // 2], BF16, tag="za")
    nc.vector.tensor_add(za[:, :], exp_buf[:, 0:Q], exp_buf[:, Q:2 * Q])
    zb = seq.tile([IL, HALF // 2], BF16, tag="zb")
    nc.vector.tensor_add(zb[:, :], exp_buf[:, 2 * Q:3 * Q],
                         exp_buf[:, 3 * Q:4 * Q])
    z2 = seq.tile([IL, HALF // 2], BF16, tag="z2")
    nc.vector.tensor_add(z2[:, :], za[:, :], zb[:, :])
    z3 = seq.tile([IL, HALF // 4], BF16, tag="z3")
    nc.vector.tensor_add(z3[:, :], z2[:, 0:HALF // 4], z2[:, HALF // 4:HALF // 2])
    zbuf = seq.tile([IL, IH * B], F32, tag="z")
    nc.vector.tensor_add(zbuf[:, :], z3[:, 0:IH * B], z3[:, IH * B:2 * IH * B])
    rz = seq.tile([IL, IH * B], F32, tag="rz")
    nc.vector.reciprocal_approx_fast(out=rz[:, :], in_=zbuf[:, :])
    rzbf = seq.tile([IL, IH * B], BF16, tag="rzbf")
    nc.vector.tensor_copy(rzbf[:, :], rz[:, :])
    xp = seq.tile([IL, IH * E * B], BF16, tag="xp")
    nc.vector.tensor_tensor(
        xp[:, :].rearrange("p (ih e b) -> p ih e b", ih=IH, e=E),
        xbf[:, :].rearrange("p (ih e b) -> p ih e b", ih=IH, e=E),
        rzbf[:, :].rearrange("p (ih b) -> p ih b", ih=IH)
        .unsqueeze(2).broadcast_to((IL, IH, E, B)),
        op=mybir.AluOpType.mult,
    )
    for op_ in range(O // 2):
        if op_ == 4 and half_cb is not None:
            half_cb()
        xc = pool.tile([IL, 2 * IH * E * B], BF16, tag="xc")
        nc.vector.tensor_tensor(
            xc[:, :].rearrange("p (oo ih e b) -> p oo ih e b",
                               oo=2, ih=IH, e=E),
            exp_buf[:, :].rearrange("p (o ih b) -> p o ih b", o=O, ih=IH)
            [:, 2 * op_:2 * op_ + 2]
            .unsqueeze(3).broadcast_to((IL, 2, IH, E, B)),
            xp[:, :].rearrange("p (ih e b) -> p ih e b", ih=IH, e=E)
            .unsqueeze(1).broadcast_to((IL, 2, IH, E, B)),
            op=mybir.AluOpType.mult,
        )
        for oo in range(2):
            o = op_ * 2 + oo
            s_ps = s_ps_list[o // 8]
            n_k = IH * E
            kt = 0
            for ih in range(IH):
                for e in range(E):
                    lhsT = xc[:, :].rearrange(
                        "p (oo ih e b) -> p oo ih e b", oo=2, ih=IH, e=E
                    )[:, oo, ih, e, :]
                    ihe = ih * E + e
                    rhs = wbf[ihe // 4][:, :].rearrange(
                        "p (k o d) -> p k o d", k=4, o=O
                    )[:, ihe % 4, o, :]
                    nc.tensor.matmul(
                        s_ps[:, (o % 8) * D:(o % 8 + 1) * D], lhsT, rhs,
                        start=(kt == 0), stop=(kt == n_k - 1),
                    )
                    kt += 1


def build():
    nc = bacc.Bacc("TRN2", target_bir_lowering=False, debug=False,
                   enable_asserts=True, num_devices=N_CORES)

    # per-core inputs (host pre-arranged; see kernel())
    xbf_d = nc.dram_tensor("xbf", [IL, IH * E * B], BF16,
                           kind="ExternalInput").ap()
    wbf_d = nc.dram_tensor("wbf", [IL, IH * E * O * D], BF16,
                           kind="ExternalInput").ap()
    wdt_d = nc.dram_tensor("wdt", [128, 4 * IH * E * IL], BF16,
                           kind="ExternalInput").ap()
    ident_d = nc.dram_tensor("ident", [128, 128], F32,
                             kind="ExternalInput").ap()

    sp_out = nc.dram_tensor("sp", [B, O * D], F32, kind="ExternalOutput").ap()

    cc0_in = nc.dram_tensor("cc0_in", [1, 16], F32)
    cc0_out = nc.dram_tensor("cc0_out", [1, 16], F32, addr_space="Shared")
    cc1_in = nc.dram_tensor("cc1_in", [B, O * D], BF16)
    cc1_out = nc.dram_tensor("cc1_out", [B, O * D], BF16, addr_space="Shared")
    cc2_in = nc.dram_tensor("cc2_in", [B, O * D], BF16)
    cc2_out = nc.dram_tensor("cc2_out", [B, O * D], BF16, addr_space="Shared")

    rg = [list(range(N_CORES))]

    with tile.TileContext(nc) as tc:
        with (
            tc.tile_pool(name="const", bufs=1) as cpool,
            tc.tile_pool(name="work", bufs=4) as pool,
            tc.tile_pool(name="psum", bufs=2, space="PSUM") as psum_pool,
            tc.tile_pool(name="seq", bufs=1) as seq_pool,
        ):
            # ---- load inputs (split across DMA queues: one dma_start
            # rides one queue, so a monolithic 1MB wbf load serializes
            # ~7us on a single queue while 12 queues idle) ----
            WQ = IH * E * O * D // 4
            XQ = IH * E * B // 2
            xbf = cpool.tile([IL, IH * E * B], BF16)
            for _q in range(2):
                nc.sync.dma_start(out=xbf[:, _q * XQ:(_q + 1) * XQ],
                                  in_=xbf_d[:, _q * XQ:(_q + 1) * XQ])
            wbf_qs = []
            for _q in range(4):
                wq = cpool.tile([IL, WQ], BF16, tag=f"wbfq{_q}")
                nc.sync.dma_start(out=wq[:, :],
                                  in_=wbf_d[:, _q * WQ:(_q + 1) * WQ])
                wbf_qs.append(wq)
            wbf = wbf_qs
            ident = cpool.tile([128, 128], F32)
            nc.sync.dma_start(out=ident[:, :], in_=ident_d)
            wdt = cpool.tile([128, 4 * IH * E * IL], BF16)
            WDQ = 4 * IH * E * IL // 2
            for _q in range(2):
                nc.sync.dma_start(out=wdt[:, _q * WDQ:(_q + 1) * WDQ],
                                  in_=wdt_d[:, _q * WDQ:(_q + 1) * WDQ])

            l_buf = cpool.tile([IL, O * IH * B], BF16)
            delta_buf = cpool.tile([IL, O * IH * B], BF16)
            exp_buf = cpool.tile([IL, O * IH * B], BF16)
            vpad = cpool.tile([B, 4 * 4 * 32], F32)
            nc.vector.memset(vpad[:, :], 0.0)
            vT = cpool.tile([128, 4 * B], BF16)

            # pre-place the one ACT table set covering every func this
            # kernel uses (exp, ln, square, copy, identity) so the
            # compiler pass inserts no further table loads (id 6 =
            # natural_log_exp_and_others in act_info.json).
            nc.scalar.add_instruction(mybir.InstLoadActFuncSet(
                name=nc.get_next_instruction_name(),
                act_func_set_id=6, ins=[], outs=[]))

            pools = (pool, psum_pool, seq_pool)

            # ---- HAM warm-up on ident (lands before the weight
            # quarters) so the s1 chain runs at 2.4GHz ----
            for _w in range(32):
                warm_ps = psum_pool.tile([128, 128], F32, tag="ps")
                nc.tensor.matmul(warm_ps[:, :], ident[:, :], ident[:, :],
                                 start=True, stop=True)

            # ---- iteration 1: uniform c -> s1 = (1/16) x @ W ----
            _sid_s1, _ = nc.enter_named_scope("s1", False)
            s_ps1 = psum_pool.tile([B, O * D], F32, tag="ps")
            kt = 0
            for ih in range(IH):
                for e in range(E):
                    ihe = ih * E + e
                    lhsT = xbf[:, :].rearrange(
                        "p (ih e b) -> p ih e b", ih=IH, e=E
                    )[:, ih, e, :]
                    rhs = wbf[ihe // 4][:, :].rearrange(
                        "p (k od) -> p k od", k=4
                    )[:, ihe % 4, :]
                    nc.tensor.matmul(
                        s_ps1[:, :], lhsT, rhs,
                        start=(kt == 0), stop=(kt == IH * E - 1),
                    )
                    kt += 1
            s_sb1 = cpool.tile([B, O * D], BF16)
            nc.scalar.mul(s_sb1[:, :], s_ps1[:, :], 1.0 / O)
            nc.sync.dma_start(out=cc1_in[:], in_=s_sb1[:, :])
            nc.leave_named_scope("s1", _sid_s1, False)
            _sid_ar1, _ = nc.enter_named_scope("ar1", False)
            nc.gpsimd.collective_compute(
                "AllReduce", mybir.AluOpType.add, replica_groups=rg,
                ins=[cc1_in[:]], outs=[cc1_out[:]],
            )
            sfull1 = cpool.tile([B, O * D], BF16)
            nc.sync.dma_start(out=sfull1[:, :], in_=cc1_out[:])
            nc.leave_named_scope("ar1", _sid_ar1, False)
            _sid_squash1, _ = nc.enter_named_scope("squash1", False)
            for _w in range(16):
                warm_ps = psum_pool.tile([64, 256], F32, tag="ps")
                nc.tensor.matmul(warm_ps[:, :], sfull1[:, 0:64],
                                 sfull1[:, :], start=True, stop=True)
            _emit_squash(nc, cpool, sfull1, vpad, tag="1")
            _emit_transpose_v(nc, psum_pool, cpool, vpad, vT, ident, tag="1")
            nc.leave_named_scope("squash1", _sid_squash1, False)

            # ---- iteration 2 ----
            _sid_ul2, _ = nc.enter_named_scope("ul2", False)
            _emit_iteration_ul(nc, tc, pools, vT, l_buf, delta_buf, wdt, xbf, 2,
                               exp_buf)
            nc.leave_named_scope("ul2", _sid_ul2, False)
            _sid_xcs2, _ = nc.enter_named_scope("xcs2", False)
            s_ps2a = psum_pool.tile([B, 8 * D], F32, tag="ps")
            s_ps2b = psum_pool.tile([B, 8 * D], F32, tag="ps")
            s_sb2 = cpool.tile([B, O * D], BF16)

            def _half_a_out():
                nc.scalar.copy(s_sb2[:, 0:8 * D], s_ps2a[:, :])
                nc.sync.dma_start(out=cc2_in[:, 0:8 * D],
                                  in_=s_sb2[:, 0:8 * D])

            _emit_softmax_xc_s(nc, tc, pools, exp_buf, xbf, wbf,
                               [s_ps2a, s_ps2b], 2, half_cb=_half_a_out)
            nc.scalar.copy(s_sb2[:, 8 * D:O * D], s_ps2b[:, :])
            nc.sync.dma_start(out=cc2_in[:, 8 * D:O * D],
                              in_=s_sb2[:, 8 * D:O * D])
            nc.leave_named_scope("xcs2", _sid_xcs2, False)
            _sid_ar2, _ = nc.enter_named_scope("ar2", False)
            nc.gpsimd.collective_compute(
                "AllReduce", mybir.AluOpType.add, replica_groups=rg,
                ins=[cc2_in[:]], outs=[cc2_out[:]],
            )
            sfull2 = cpool.tile([B, O * D], BF16)
            nc.sync.dma_start(out=sfull2[:, :], in_=cc2_out[:])
            nc.leave_named_scope("ar2", _sid_ar2, False)
            _sid_squash2, _ = nc.enter_named_scope("squash2", False)
            for _w in range(16):
                warm_ps = psum_pool.tile([64, 256], F32, tag="ps")
                nc.tensor.matmul(warm_ps[:, :], sfull2[:, 0:64],
                                 sfull2[:, :], start=True, stop=True)
            _emit_squash(nc, cpool, sfull2, vpad, tag="2")
            _emit_transpose_v(nc, psum_pool, cpool, vpad, vT, ident, tag="2")
            nc.leave_named_scope("squash2", _sid_squash2, False)

            # ---- iteration 3 (final: partial s3 out, host finishes) ----
            _sid_ul3, _ = nc.enter_named_scope("ul3", False)
            _emit_iteration_ul(nc, tc, pools, vT, l_buf, delta_buf, wdt, xbf, 3,
                               exp_buf)
            nc.leave_named_scope("ul3", _sid_ul3, False)
            _sid_xcs3, _ = nc.enter_named_scope("xcs3", False)
            s_ps3a = psum_pool.tile([B, 8 * D], F32, tag="ps")
            s_ps3b = psum_pool.tile([B, 8 * D], F32, tag="ps")
            sp_sb = cpool.tile([B, O * D], F32)

            def _half_a_sp():
                nc.scalar.copy(sp_sb[:, 0:8 * D], s_ps3a[:, :])
                nc.sync.dma_start(out=sp_out[:, 0:8 * D],
                                  in_=sp_sb[:, 0:8 * D])

            _emit_softmax_xc_s(nc, tc, pools, exp_buf, xbf, wbf,
                               [s_ps3a, s_ps3b], 3, half_cb=_half_a_sp)
            nc.leave_named_scope("xcs3", _sid_xcs3, False)
            nc.scalar.copy(sp_sb[:, 8 * D:O * D], s_ps3b[:, :])
            nc.sync.dma_start(out=sp_out[:, 8 * D:O * D],
                              in_=sp_sb[:, 8 * D:O * D])

    nc.compile()
    return nc


def _host_prep(x, weight):
    """Build the per-core input maps (free host-side rearrangement)."""
    in_maps = []
    ident = np.eye(128, dtype=np.float32)
    for c in range(N_CORES):
        x_c = x[:, c * IC:(c + 1) * IC, :]          # [B, 256, E]
        w_c = weight[:, c * IC:(c + 1) * IC, :, :]  # [O, 256, D, E]

        # xt [il, (ih, e, b)]
        xr = x_c.reshape(B, IH, IL, E)              # b, ih, il, e
        xt = np.ascontiguousarray(
            xr.transpose(2, 1, 3, 0)                # il, ih, e, b
        ).reshape(IL, IH * E * B)

        # w [il, (ih, e, h, g, d)] with o = 4h + g
        wr = w_c.reshape(4, 4, IH, IL, D, E)        # h, g, ih, il, d, e
        w_f = np.ascontiguousarray(
            wr.transpose(3, 2, 5, 0, 1, 4)          # il, ih, e, h, g, d
        ).reshape(IL, IH * E * O * D)

        # wdt [(g, dd=32), (h, ih, e, il)] (dd >= 16 zero)
        wdtv = np.zeros((4, 32, 4, IH, E, IL), dtype=np.float32)
        wdtv[:, :D] = wr.transpose(1, 4, 0, 2, 5, 3)  # g, d, h, ih, e, il
        wdt = wdtv.reshape(128, 4 * IH * E * IL)

        in_maps.append({
            "xbf": xt.astype(NPBF16),
            "wbf": w_f.astype(NPBF16),
            "wdt": wdt.astype(NPBF16),
            "ident": ident,
        })
    return in_maps


def _host_finish(partials):
    """Sum the 8 per-core partial s3 tensors, final squash (the unshard)."""
    s = np.zeros((B, O * D), dtype=np.float64)
    for p in partials:
        s += p.astype(np.float64)
    s = s.reshape(B, O, D)
    n2 = (s * s).sum(axis=-1, keepdims=True)
    n = np.sqrt(n2)
    v = (n2 / (1.0 + n2) / (n + EPS)) * s
    return v.astype(np.float32)


def kernel(x, weight, _trace=False):
    x = np.asarray(x, dtype=np.float32)
    weight = np.asarray(weight, dtype=np.float32)
    if "nc" not in _CACHE:
        _CACHE["nc"] = build()
    nc = _CACHE["nc"]
    in_maps = _host_prep(x, weight)
    res = run_bass_kernel_spmd(
        nc, in_maps, core_ids=list(range(N_CORES)), trace=_trace
    )
    out = _host_finish([res.results[c]["sp"] for c in range(N_CORES)])
    if _trace:
        _CACHE["last_result"] = res
    return out


if __name__ == "__main__":
    rng = np.random.default_rng(0)
    x = rng.standard_normal((B, I_FULL, E)).astype(np.float32)
    w = (0.01 * rng.standard_normal((O, I_FULL, D, E))).astype(np.float32)
    out = kernel(x, w)
    print("out", out.shape, out.dtype, np.abs(out).max())


# revision 17
# speedup vs baseline: 1.0824x; 1.0824x over previous
"""Trainium2 Bass kernel for DenseCapsule dynamic routing (3 iterations).

Problem: x[128,2048,8] f32, weight[16,2048,16,8] f32 -> out[128,16,16] f32.
  x_hat = einsum('oide,bie->boid', W, x); 3 routing iterations
  (softmax over o, c-weighted i-sum, squash, agreement update).

Strategy (8 NeuronCores, shard in_num_caps I=2048 -> 256 per core):
  x_hat is never materialized. Per iteration, everything factors through W:
    iter1: c uniform -> s1 = (1/16) * [x @ W]   (one matmul chain)
    u = v . W (PE), l = sum_e x*u (DVE), softmax (ACT/DVE),
    xc = c*x (DVE), s = xc @ W (PE).
  Cross-core: AllReduce of partial s ([128,16,16] f32) after iters 1 and 2;
  the final iteration's partial s is returned per-core and the host does the
  gather-sum + final squash (that is the "unshard" step).

Performance changes vs the first working version (in-kernel 196 -> 176.9us,
per-core pre-AllReduce path 27.6 -> 21.5us; measured total also includes a
30-63us run-variable launch-skew gate at the first AllReduce -- the profiled
core waits for the last-dispatched core):
  - single ACT table set: sqrt(n2) = exp(0.5*ln(n2)) and a pre-placed
    InstLoadActFuncSet(natural_log_exp_and_others) -> 1 table load
    instead of 5 (1.28us each, 3 on the critical path).
  - input loads split across DMA queues (one dma_start rides one queue;
    the monolithic 1MB wbf load was a 7.3us single-queue serial tail).
  - HAM warm-up matmuls gated on each AllReduce result fill the squash
    windows so the ul chains run at 2.4GHz instead of the cold 1.2GHz.
  - softmax exp quarters emitted inside the ul loop (ACT queue is FIFO:
    emitted after the loop they queued behind all 16 u-evacuations).
  - bf16 AllReduce payloads (comms 14.8->12.4us, 13.5->10.6us).
  - balanced bf16 Z-tree (5 ops) instead of split f32 trees (~9 ops);
    reciprocal_approx_fast for 1/Z.
  - xu/tree/xc DVE ops batched over o-pairs; pair 0 split per-o so the
    first xu starts after the first PSUM evacuation.
  - iter-3 logit accumulation (b += l) folded per-pair into the tree flow.
  - s-halves evacuated + DMA'd as soon as o0-7 complete (earlier
    AllReduce trigger / output streaming).

Layout conventions per core (SBUF partition dim first):
  i_local = ih*128 + il  (ih in {0,1}, il = partition 0..127)
  o = 4*h + g            (g in 0..3 selects a 32-partition group, h in 0..3)
  d padded to 32 rows (dd) for the u-matmul stationary operand.
"""

import sys

for _p in ("/opt/trn_rl_repo", "/root/.axon_site/_ro/trn_rl_repo"):
    if _p not in sys.path:
        sys.path.insert(0, _p)

import numpy as np
import ml_dtypes

import concourse.bass as bass
import concourse.bacc as bacc
import concourse.mybir as mybir
import concourse.tile as tile
from concourse.bass_utils import run_bass_kernel_spmd

F32 = mybir.dt.float32
BF16 = mybir.dt.bfloat16
NPBF16 = ml_dtypes.bfloat16
EXP = mybir.ActivationFunctionType.Exp
LN = mybir.ActivationFunctionType.Ln
SQUARE = mybir.ActivationFunctionType.Square

N_CORES = 8
B = 128          # batch
I_FULL = 2048    # in caps
IC = 256         # in caps per core
IL = 128         # partition dim of i
IH = IC // IL    # 2
E = 8            # in cap dim
O = 16           # out caps
D = 16           # out cap dim
EPS = 1e-8

_CACHE = {}


def _emit_squash(nc, pool, sfull, vpad, tag):
    """squash on [(b)=128, (h,g,d)=256] f32 layout; writes v into vpad
    ([(b), (h,g,dd=32)=512] f32, pad rows stay zero).

    scale = n2/((1+n2)(n+eps)) ~= n/(1+n2) with n = sqrt(n2); sqrt is
    computed as exp(0.5*ln(n2)) to stay inside one ACT table set."""
    sq = pool.tile([B, O * D], F32, tag="sq")
    nc.scalar.square(sq[:, :], sfull[:, :])
    nrm2 = pool.tile([B, O], F32, tag="nrm2")
    # reduce innermost d (16) of (o=16, d=16)
    nc.vector.reduce_sum(
        nrm2[:, :],
        sq[:, :].rearrange("p (o d) -> p o d", d=D),
        axis=mybir.AxisListType.X,
    )
    # n = sqrt(n2) = exp(0.5 * ln(n2)); ln(0) = -inf -> exp(-inf) = 0  (ok)
    lnn = pool.tile([B, O], F32, tag="lnn")
    nc.scalar.activation(lnn[:, :], nrm2[:, :], LN)
    q = pool.tile([B, O], F32, tag="q")
    nc.scalar.activation(q[:, :], lnn[:, :], EXP, scale=0.5)
    t1 = pool.tile([B, O], F32, tag="t1")
    nc.vector.tensor_scalar_add(t1[:, :], nrm2[:, :], 1.0)
    rden = pool.tile([B, O], F32, tag="rden")
    nc.vector.reciprocal(rden[:, :], t1[:, :])
    scale = pool.tile([B, O], F32, tag="scale")
    nc.vector.tensor_mul(scale[:, :], q[:, :], rden[:, :])
    # v = s * scale (broadcast over d) into vpad[(b), (h, g, dd<16)]
    s_v = sfull[:, :].rearrange("p (h g d) -> p h g d", h=4, g=4)
    scale_v = scale[:, :].rearrange("p (h g) -> p h g", h=4).broadcast_to(
        (B, 4, 4, D)
    )
    vslice = vpad[:, :].rearrange("p (h g dd) -> p h g dd", h=4, g=4)[:, :, :, 0:D]
    nc.vector.tensor_tensor(vslice, s_v, scale_v, op=mybir.AluOpType.mult)


def _emit_transpose_v(nc, psum_pool, pool, vpad, vT, ident, tag):
    """vpad [(b), (h, g, dd)=512] f32 -> vT [(g,dd)=128, (h,b)=512] bf16
    via 4 PE transposes (one per h) + ACT evacuations."""
    for h in range(4):
        tp = psum_pool.tile([128, B], F32, tag="ps")
        in_slice = vpad[:, h * 128:(h + 1) * 128]
        nc.tensor.transpose(tp[:, :], in_slice, ident[:, :])
        nc.scalar.copy(vT[:, h * B:(h + 1) * B], tp[:, :])


def _emit_iteration_ul(nc, tc, pools, vT, l_buf, delta_buf, wdt, xbf, itr,
                       exp_buf):
    """u = v.W (PE) -> evac (ACT) -> per o-PAIR: xu = x*u (DVE) ->
    e-reduction rounds (DVE) -> l (or delta for iter 3)."""
    pool, psum_pool, seq = pools
    PAIR = 2 * IH * E * B  # 4096: free extent of one o-pair
    for op_ in range(O // 2):  # o-pair index
        u2 = pool.tile([IL, PAIR], BF16, tag="u2")
        for oo in range(2):
            o = op_ * 2 + oo
            h, g = o // 4, o % 4
            u_ps = psum_pool.tile([IL, IH * E * B], F32, tag="ps")
            for ih in range(IH):
                for e in range(E):
                    lhsT = wdt[:, :].rearrange(
                        "p (h ih e il) -> p h ih e il", h=4, ih=IH, e=E
                    )[32 * g:32 * (g + 1), h, ih, e, :]
                    rhs = vT[32 * g:32 * (g + 1), h * B:(h + 1) * B]
                    nc.tensor.matmul(
                        u_ps[:, (ih * E + e) * B:(ih * E + e + 1) * B], lhsT, rhs,
                        start=True, stop=True, tile_position=(32 * g, 0),
                    )
            nc.scalar.copy(u2[:, oo * IH * E * B:(oo + 1) * IH * E * B],
                           u_ps[:, :])
        # xu = x * u; pair 0 is split per-o so the first xu starts right
        # after the first evacuation instead of waiting for both
        xu = pool.tile([IL, PAIR], BF16, tag="xu")
        HALF_P = IH * E * B
        xsub = 2 if op_ == 0 else 1
        for _s in range(xsub):
            lo = _s * (2 // xsub)
            n_oo = 2 // xsub
            nc.vector.tensor_tensor(
                xu[:, lo * HALF_P:(lo + n_oo) * HALF_P]
                .rearrange("p (oo ihe b) -> p oo ihe b", oo=n_oo, b=B),
                xbf[:, :].rearrange("p (ihe b) -> p ihe b", b=B)
                .unsqueeze(1).broadcast_to((IL, n_oo, IH * E, B)),
                u2[:, lo * HALF_P:(lo + n_oo) * HALF_P]
                .rearrange("p (oo ihe b) -> p oo ihe b", oo=n_oo, b=B),
                op=mybir.AluOpType.mult,
            )
        # e-reduction rounds within each (oo, ih) block: 8 -> 4 -> 2 -> 1
        xu4 = xu[:, :].rearrange("p (x half eb) -> p x half eb",
                                 x=2 * IH, half=2)
        r1 = pool.tile([IL, PAIR // 2], BF16, tag="r1")
        r1v = r1[:, :].rearrange("p (x eb) -> p x eb", x=2 * IH)
        for _s in range(xsub):
            sl = slice(_s * (2 * IH // xsub), (_s + 1) * (2 * IH // xsub))
            nc.vector.tensor_tensor(r1v[:, sl], xu4[:, sl, 0], xu4[:, sl, 1],
                                    op=mybir.AluOpType.add)
        r1h = r1[:, :].rearrange("p (x half eb) -> p x half eb",
                                 x=2 * IH, half=2)
        r2 = pool.tile([IL, PAIR // 4], BF16, tag="r2")
        r2v = r2[:, :].rearrange("p (x eb) -> p x eb", x=2 * IH)
        nc.vector.tensor_tensor(r2v, r1h[:, :, 0], r1h[:, :, 1],
                                op=mybir.AluOpType.add)
        r2h = r2[:, :].rearrange("p (x half b) -> p x half b",
                                 x=2 * IH, half=2)
        dst_buf = l_buf if itr == 2 else delta_buf
        dst = dst_buf[:, :].rearrange(
            "p (op x b) -> p op x b", op=O // 2, x=2 * IH
        )[:, op_]
        nc.vector.tensor_tensor(dst, r2h[:, :, 0], r2h[:, :, 1],
                                op=mybir.AluOpType.add)
        if itr == 3:
            # fold b += l per pair (pipelines; unblocks exp sooner)
            lp = l_buf[:, op_ * 2 * IH * B:(op_ + 1) * 2 * IH * B]
            dp = delta_buf[:, op_ * 2 * IH * B:(op_ + 1) * 2 * IH * B]
            nc.vector.tensor_add(lp, lp, dp)
        if op_ % 2 == 1:
            _q = op_ // 2
            QW = 4 * IH * B
            nc.scalar.activation(exp_buf[:, _q * QW:(_q + 1) * QW],
                                 l_buf[:, _q * QW:(_q + 1) * QW], EXP)


def _emit_softmax_xc_s(nc, tc, pools, exp_buf, xbf, wbf, s_ps_list, itr,
                       half_cb=None):
    """exp (ACT, halves), balanced bf16 Z-tree, 1/Z (fast recip),
    xprime = x/Z, then per o-pair xc = exp*xprime and the 16 accumulating
    s-matmuls per o into s_ps halves [(b), 8*16] each."""
    pool, psum_pool, seq = pools
    HALF = 8 * IH * B  # 2048
    Q = HALF // 2      # 1024: one exp quarter (4 o's)
    za = seq.tile([IL, HALF // 2], BF16, tag="za")
    nc.vector.tensor_add(za[:, :], exp_buf[:, 0:Q], exp_buf[:, Q:2 * Q])
    zb = seq.tile([IL, HALF // 2], BF16, tag="zb")
    nc.vector.tensor_add(zb[:, :], exp_buf[:, 2 * Q:3 * Q],
                         exp_buf[:, 3 * Q:4 * Q])
    z2 = seq.tile([IL, HALF // 2], BF16, tag="z2")
    nc.vector.tensor_add(z2[:, :], za[:, :], zb[:, :])
    z3 = seq.tile([IL, HALF // 4], BF16, tag="z3")
    nc.vector.tensor_add(z3[:, :], z2[:, 0:HALF // 4], z2[:, HALF // 4:HALF // 2])
    zbuf = seq.tile([IL, IH * B], F32, tag="z")
    nc.vector.tensor_add(zbuf[:, :], z3[:, 0:IH * B], z3[:, IH * B:2 * IH * B])
    rz = seq.tile([IL, IH * B], F32, tag="rz")
    nc.vector.reciprocal_approx_fast(out=rz[:, :], in_=zbuf[:, :])
    rzbf = seq.tile([IL, IH * B], BF16, tag="rzbf")
    nc.vector.tensor_copy(rzbf[:, :], rz[:, :])
    xp = seq.tile([IL, IH * E * B], BF16, tag="xp")
    nc.vector.tensor_tensor(
        xp[:, :].rearrange("p (ih e b) -> p ih e b", ih=IH, e=E),
        xbf[:, :].rearrange("p (ih e b) -> p ih e b", ih=IH, e=E),
        rzbf[:, :].rearrange("p (ih b) -> p ih b", ih=IH)
        .unsqueeze(2).broadcast_to((IL, IH, E, B)),
        op=mybir.AluOpType.mult,
    )
    for op_ in range(O // 2):
        if op_ == 4 and half_cb is not None:
            half_cb()
        xc = pool.tile([IL, 2 * IH * E * B], BF16, tag="xc")
        nc.vector.tensor_tensor(
            xc[:, :].rearrange("p (oo ih e b) -> p oo ih e b",
                               oo=2, ih=IH, e=E),
            exp_buf[:, :].rearrange("p (o ih b) -> p o ih b", o=O, ih=IH)
            [:, 2 * op_:2 * op_ + 2]
            .unsqueeze(3).broadcast_to((IL, 2, IH, E, B)),
            xp[:, :].rearrange("p (ih e b) -> p ih e b", ih=IH, e=E)
            .unsqueeze(1).broadcast_to((IL, 2, IH, E, B)),
            op=mybir.AluOpType.mult,
        )
        for oo in range(2):
            o = op_ * 2 + oo
            s_ps = s_ps_list[o // 8]
            n_k = IH * E
            kt = 0
            for ih in range(IH):
                for e in range(E):
                    lhsT = xc[:, :].rearrange(
                        "p (oo ih e b) -> p oo ih e b", oo=2, ih=IH, e=E
                    )[:, oo, ih, e, :]
                    ihe = ih * E + e
                    rhs = wbf[ihe // 4][:, :].rearrange(
                        "p (k o d) -> p k o d", k=4, o=O
                    )[:, ihe % 4, o, :]
                    nc.tensor.matmul(
                        s_ps[:, (o % 8) * D:(o % 8 + 1) * D], lhsT, rhs,
                        start=(kt == 0), stop=(kt == n_k - 1),
                    )
                    kt += 1


def build():
    nc = bacc.Bacc("TRN2", target_bir_lowering=False, debug=False,
                   enable_asserts=True, num_devices=N_CORES)

    # per-core inputs (host pre-arranged; see kernel())
    xbf_d = nc.dram_tensor("xbf", [IL, IH * E * B], BF16,
                           kind="ExternalInput").ap()
    wbf_d = nc.dram_tensor("wbf", [IL, IH * E * O * D], BF16,
                           kind="ExternalInput").ap()
    wdt_d = nc.dram_tensor("wdt", [128, 4 * IH * E * IL], BF16,
                           kind="ExternalInput").ap()
    ident_d = nc.dram_tensor("ident", [128, 128], F32,
                             kind="ExternalInput").ap()

    sp_out = nc.dram_tensor("sp", [B, O * D], F32, kind="ExternalOutput").ap()

    cc0_in = nc.dram_tensor("cc0_in", [1, 16], F32)
    cc0_out = nc.dram_tensor("cc0_out", [1, 16], F32, addr_space="Shared")
    cc1_in = nc.dram_tensor("cc1_in", [B, O * D], BF16)
    cc1_out = nc.dram_tensor("cc1_out", [B, O * D], BF16, addr_space="Shared")
    cc2_in = nc.dram_tensor("cc2_in", [B, O * D], BF16)
    cc2_out = nc.dram_tensor("cc2_out", [B, O * D], BF16, addr_space="Shared")

    rg = [list(range(N_CORES))]

    with tile.TileContext(nc) as tc:
        with (
            tc.tile_pool(name="const", bufs=1) as cpool,
            tc.tile_pool(name="work", bufs=4) as pool,
            tc.tile_pool(name="psum", bufs=2, space="PSUM") as psum_pool,
            tc.tile_pool(name="seq", bufs=1) as seq_pool,
        ):
            # ---- load inputs (split across DMA queues: one dma_start
            # rides one queue, so a monolithic 1MB wbf load serializes
            # ~7us on a single queue while 12 queues idle) ----
            WQ = IH * E * O * D // 4
            XQ = IH * E * B // 2
            xbf = cpool.tile([IL, IH * E * B], BF16)
            for _q in range(2):
                nc.sync.dma_start(out=xbf[:, _q * XQ:(_q + 1) * XQ],
                                  in_=xbf_d[:, _q * XQ:(_q + 1) * XQ])
            wbf_qs = []
            for _q in range(4):
                wq = cpool.tile([IL, WQ], BF16, tag=f"wbfq{_q}")
                nc.sync.dma_start(out=wq[:, :],
                                  in_=wbf_d[:, _q * WQ:(_q + 1) * WQ])
                wbf_qs.append(wq)
            wbf = wbf_qs
            ident = cpool.tile([128, 128], F32)
            nc.sync.dma_start(out=ident[:, :], in_=ident_d)
            wdt = cpool.tile([128, 4 * IH * E * IL], BF16)
            WDQ = 4 * IH * E * IL // 2
            for _q in range(2):
                nc.sync.dma_start(out=wdt[:, _q * WDQ:(_q + 1) * WDQ],
                                  in_=wdt_d[:, _q * WDQ:(_q + 1) * WDQ])

            l_buf = cpool.tile([IL, O * IH * B], BF16)
            delta_buf = cpool.tile([IL, O * IH * B], BF16)
            exp_buf = cpool.tile([IL, O * IH * B], BF16)
            vpad = cpool.tile([B, 4 * 4 * 32], F32)
            nc.vector.memset(vpad[:, :], 0.0)
            vT = cpool.tile([128, 4 * B], BF16)

            # pre-place the one ACT table set covering every func this
            # kernel uses (exp, ln, square, copy, identity) so the
            # compiler pass inserts no further table loads (id 6 =
            # natural_log_exp_and_others in act_info.json).
            nc.scalar.add_instruction(mybir.InstLoadActFuncSet(
                name=nc.get_next_instruction_name(),
                act_func_set_id=6, ins=[], outs=[]))

            pools = (pool, psum_pool, seq_pool)

            # ---- iteration 1: uniform c -> s1 = (1/16) x @ W ----
            _sid_s1, _ = nc.enter_named_scope("s1", False)
            s_ps1 = psum_pool.tile([B, O * D], F32, tag="ps")
            kt = 0
            for ih in range(IH):
                for e in range(E):
                    ihe = ih * E + e
                    lhsT = xbf[:, :].rearrange(
                        "p (ih e b) -> p ih e b", ih=IH, e=E
                    )[:, ih, e, :]
                    rhs = wbf[ihe // 4][:, :].rearrange(
                        "p (k od) -> p k od", k=4
                    )[:, ihe % 4, :]
                    nc.tensor.matmul(
                        s_ps1[:, :], lhsT, rhs,
                        start=(kt == 0), stop=(kt == IH * E - 1),
                    )
                    kt += 1
            s_sb1 = cpool.tile([B, O * D], BF16)
            nc.scalar.mul(s_sb1[:, :], s_ps1[:, :], 1.0 / O)
            nc.sync.dma_start(out=cc1_in[:], in_=s_sb1[:, :])
            nc.leave_named_scope("s1", _sid_s1, False)
            _sid_ar1, _ = nc.enter_named_scope("ar1", False)
            nc.gpsimd.collective_compute(
                "AllReduce", mybir.AluOpType.add, replica_groups=rg,
                ins=[cc1_in[:]], outs=[cc1_out[:]],
            )
            sfull1 = cpool.tile([B, O * D], BF16)
            nc.sync.dma_start(out=sfull1[:, :], in_=cc1_out[:])
            nc.leave_named_scope("ar1", _sid_ar1, False)
            _sid_squash1, _ = nc.enter_named_scope("squash1", False)
            for _w in range(16):
                warm_ps = psum_pool.tile([64, 256], F32, tag="ps")
                nc.tensor.matmul(warm_ps[:, :], sfull1[:, 0:64],
                                 sfull1[:, :], start=True, stop=True)
            _emit_squash(nc, cpool, sfull1, vpad, tag="1")
            _emit_transpose_v(nc, psum_pool, cpool, vpad, vT, ident, tag="1")
            nc.leave_named_scope("squash1", _sid_squash1, False)

            # ---- iteration 2 ----
            _sid_ul2, _ = nc.enter_named_scope("ul2", False)
            _emit_iteration_ul(nc, tc, pools, vT, l_buf, delta_buf, wdt, xbf, 2,
                               exp_buf)
            nc.leave_named_scope("ul2", _sid_ul2, False)
            _sid_xcs2, _ = nc.enter_named_scope("xcs2", False)
            s_ps2a = psum_pool.tile([B, 8 * D], F32, tag="ps")
            s_ps2b = psum_pool.tile([B, 8 * D], F32, tag="ps")
            s_sb2 = cpool.tile([B, O * D], BF16)

            def _half_a_out():
                nc.scalar.copy(s_sb2[:, 0:8 * D], s_ps2a[:, :])
                nc.sync.dma_start(out=cc2_in[:, 0:8 * D],
                                  in_=s_sb2[:, 0:8 * D])

            _emit_softmax_xc_s(nc, tc, pools, exp_buf, xbf, wbf,
                               [s_ps2a, s_ps2b], 2, half_cb=_half_a_out)
            nc.scalar.copy(s_sb2[:, 8 * D:O * D], s_ps2b[:, :])
            nc.sync.dma_start(out=cc2_in[:, 8 * D:O * D],
                              in_=s_sb2[:, 8 * D:O * D])
            nc.leave_named_scope("xcs2", _sid_xcs2, False)
            _sid_ar2, _ = nc.enter_named_scope("ar2", False)
            nc.gpsimd.collective_compute(
                "AllReduce", mybir.AluOpType.add, replica_groups=rg,
                ins=[cc2_in[:]], outs=[cc2_out[:]],
            )
            sfull2 = cpool.tile([B, O * D], BF16)
            nc.sync.dma_start(out=sfull2[:, :], in_=cc2_out[:])
            nc.leave_named_scope("ar2", _sid_ar2, False)
            _sid_squash2, _ = nc.enter_named_scope("squash2", False)
            for _w in range(16):
                warm_ps = psum_pool.tile([64, 256], F32, tag="ps")
                nc.tensor.matmul(warm_ps[:, :], sfull2[:, 0:64],
                                 sfull2[:, :], start=True, stop=True)
            _emit_squash(nc, cpool, sfull2, vpad, tag="2")
            _emit_transpose_v(nc, psum_pool, cpool, vpad, vT, ident, tag="2")
            nc.leave_named_scope("squash2", _sid_squash2, False)

            # ---- iteration 3 (final: partial s3 out, host finishes) ----
            _sid_ul3, _ = nc.enter_named_scope("ul3", False)
            _emit_iteration_ul(nc, tc, pools, vT, l_buf, delta_buf, wdt, xbf, 3,
                               exp_buf)
            nc.leave_named_scope("ul3", _sid_ul3, False)
            _sid_xcs3, _ = nc.enter_named_scope("xcs3", False)
            s_ps3a = psum_pool.tile([B, 8 * D], F32, tag="ps")
            s_ps3b = psum_pool.tile([B, 8 * D], F32, tag="ps")
            sp_sb = cpool.tile([B, O * D], F32)

            def _half_a_sp():
                nc.scalar.copy(sp_sb[:, 0:8 * D], s_ps3a[:, :])
                nc.sync.dma_start(out=sp_out[:, 0:8 * D],
                                  in_=sp_sb[:, 0:8 * D])

            _emit_softmax_xc_s(nc, tc, pools, exp_buf, xbf, wbf,
                               [s_ps3a, s_ps3b], 3, half_cb=_half_a_sp)
            nc.leave_named_scope("xcs3", _sid_xcs3, False)
            nc.scalar.copy(sp_sb[:, 8 * D:O * D], s_ps3b[:, :])
            nc.sync.dma_start(out=sp_out[:, 8 * D:O * D],
                              in_=sp_sb[:, 8 * D:O * D])

    nc.compile()
    return nc


def _host_prep(x, weight):
    """Build the per-core input maps (free host-side rearrangement)."""
    in_maps = []
    ident = np.eye(128, dtype=np.float32)
    for c in range(N_CORES):
        x_c = x[:, c * IC:(c + 1) * IC, :]          # [B, 256, E]
        w_c = weight[:, c * IC:(c + 1) * IC, :, :]  # [O, 256, D, E]

        # xt [il, (ih, e, b)]
        xr = x_c.reshape(B, IH, IL, E)              # b, ih, il, e
        xt = np.ascontiguousarray(
            xr.transpose(2, 1, 3, 0)                # il, ih, e, b
        ).reshape(IL, IH * E * B)

        # w [il, (ih, e, h, g, d)] with o = 4h + g
        wr = w_c.reshape(4, 4, IH, IL, D, E)        # h, g, ih, il, d, e
        w_f = np.ascontiguousarray(
            wr.transpose(3, 2, 5, 0, 1, 4)          # il, ih, e, h, g, d
        ).reshape(IL, IH * E * O * D)

        # wdt [(g, dd=32), (h, ih, e, il)] (dd >= 16 zero)
        wdtv = np.zeros((4, 32, 4, IH, E, IL), dtype=np.float32)
        wdtv[:, :D] = wr.transpose(1, 4, 0, 2, 5, 3)  # g, d, h, ih, e, il
        wdt = wdtv.reshape(128, 4 * IH * E * IL)

        in_maps.append({
            "xbf": xt.astype(NPBF16),
            "wbf": w_f.astype(NPBF16),
            "wdt": wdt.astype(NPBF16),
            "ident": ident,
        })
    return in_maps


def _host_finish(partials):
    """Sum the 8 per-core partial s3 tensors, final squash (the unshard)."""
    s = np.zeros((B, O * D), dtype=np.float64)
    for p in partials:
        s += p.astype(np.float64)
    s = s.reshape(B, O, D)
    n2 = (s * s).sum(axis=-1, keepdims=True)
    n = np.sqrt(n2)
    v = (n2 / (1.0 + n2) / (n + EPS)) * s
    return v.astype(np.float32)


def kernel(x, weight, _trace=False):
    x = np.asarray(x, dtype=np.float32)
    weight = np.asarray(weight, dtype=np.float32)
    if "nc" not in _CACHE:
        _CACHE["nc"] = build()
    nc = _CACHE["nc"]
    in_maps = _host_prep(x, weight)
    res = run_bass_kernel_spmd(
        nc, in_maps, core_ids=list(range(N_CORES)), trace=_trace
    )
    out = _host_finish([res.results[c]["sp"] for c in range(N_CORES)])
    if _trace:
        _CACHE["last_result"] = res
    return out


if __name__ == "__main__":
    rng = np.random.default_rng(0)
    x = rng.standard_normal((B, I_FULL, E)).astype(np.float32)
    w = (0.01 * rng.standard_normal((O, I_FULL, D, E))).astype(np.float32)
    out = kernel(x, w)
    print("out", out.shape, out.dtype, np.abs(out).max())
